# revision 12
# baseline (speedup 1.0000x reference)
"""Bahdanau additive attention on 8 Trainium2 NeuronCores.

reference:
    eh = enc @ W_h.T            [B,S,H]
    qs = q   @ W_s.T            [B,T,H]
    score[b,t,s] = sum_h v[h] * tanh(eh[b,s,h] + qs[b,t,h])
    attn = softmax_s(score); ctx[b,t,:] = sum_s attn[b,t,s] * enc[b,s,:]

Sharding: data-parallel over batch B=8, one batch per NeuronCore.

Algorithm (replaces the 33.5M-element/core direct tanh evaluation):
separable Fourier approximation
    tanh(x) ~= sum_{k=1..6} c_k sin(k*w*x),  w = pi/9
so with angle addition the score becomes a plain contraction
    score[t,s] = sum_h sum_k (c_k v_h sin_k(eh)) cos_k(qs)
                            + (c_k v_h cos_k(eh)) sin_k(qs)
Per-core device program:
  - PE: ehT/qsT projections (fp16 weights, fp32 PSUM)
  - ACT: base pair sin(w/2 x), cos(w/2 x) (in the [-pi,pi] spline domain)
    plus squares for the double-angle cosines
  - DVE: Chebyshev/double-angle ladder for harmonics k=1..6 (fp16),
    then per-(k,phase,dc) folds multiplying by c_k*v (per-partition scalars)
  - PE: score matmul, contraction (h,k,phase) = 12*512 on partitions
  - softmax without exp: t = tanh(score/2) on ACT, e^z = (1+t)/(1-t) via
    reciprocal_approx_fast on DVE; normalization folded into the epilogue
  - PE transposes attn, context matmul against enc natural layout
"""

import sys

try:
    import concourse.bass as bass  # noqa: F401
except ImportError:  # pragma: no cover
    sys.path.insert(0, "/opt/trn_rl_repo")

import numpy as np

import concourse.bass as bass
import concourse.bacc as bacc
import concourse.mybir as mybir
from concourse import tile

FP32 = mybir.dt.float32
FP16 = mybir.dt.float16

N_CORES = 8
H = 512
T_FULL = 256
S_FULL = 256

K_HARM = 6
L_PER = 9.0
OMEGA = np.pi / L_PER


def design_coeffs():
    """Weighted LSQ fit of tanh(x) by sum_k c_k sin(k*OMEGA*x)."""
    xs = np.linspace(0, L_PER, 40001)
    wgt = np.exp(-xs ** 2 / 4.0)
    wgt = np.maximum(wgt, 1e-3)
    wgt[xs > 0.97 * L_PER] *= 0.05
    A = np.stack([np.sin(k * OMEGA * xs) for k in range(1, K_HARM + 1)], 1)
    Wt = np.sqrt(wgt)
    c, *_ = np.linalg.lstsq(A * Wt[:, None], np.tanh(xs) * Wt, rcond=None)
    return c


C_K = design_coeffs()

MULT = mybir.AluOpType.mult
ADD = mybir.AluOpType.add
SUB = mybir.AluOpType.subtract


def build_program(T=T_FULL, S=S_FULL, n_cores=N_CORES, nrep=1, debug=False,
                  unroll=1):
    assert H % 128 == 0 and T % 128 == 0 and S % 128 == 0
    DC = H // 128
    TB = T // 128
    SB = S // 128
    NF = 2 * K_HARM          # folded feature tiles (k, sin/cos)
    W2 = S + T               # combined free width of one feature tile

    nc = bacc.Bacc("TRN2", target_bir_lowering=False, debug=debug,
                   num_devices=n_cores)

    encT_d = nc.dram_tensor("encT16", [H, S], FP16, kind="ExternalInput")
    qT_d = nc.dram_tensor("qT16", [H, T], FP16, kind="ExternalInput")
    whT_d = nc.dram_tensor("whT16", [H, H], FP16, kind="ExternalInput")
    wsT_d = nc.dram_tensor("wsT16", [H, H], FP16, kind="ExternalInput")
    enc_d = nc.dram_tensor("enc32", [S, H], FP32, kind="ExternalInput")
    vc_d = nc.dram_tensor("vc32", [128, NF * DC + 1], FP32, kind="ExternalInput")
    ident_d = nc.dram_tensor("ident", [128, 128], FP32, kind="ExternalInput")
    ctx_d = nc.dram_tensor("ctx", [T, H], FP32, kind="ExternalOutput")

    with tile.TileContext(nc) as tc:
        with (
            tc.tile_pool(name="const", bufs=1) as const_pool,
            tc.tile_pool(name="xcat", bufs=2) as xcat_pool,
            tc.tile_pool(name="feat", bufs=1) as feat_pool,
            tc.tile_pool(name="fold", bufs=1) as fold_pool,
            tc.tile_pool(name="post", bufs=2) as post_pool,
            tc.tile_pool(name="ppsum", bufs=2, space=bass.MemorySpace.PSUM) as ppsum,
            tc.tile_pool(name="spsum", bufs=2, space=bass.MemorySpace.PSUM) as spsum,
            tc.tile_pool(name="apsum", bufs=1, space=bass.MemorySpace.PSUM) as apsum,
            tc.tile_pool(name="cpsum", bufs=1, space=bass.MemorySpace.PSUM) as cpsum,
        ):
            # ---- constants / inputs (outside the timed loop) ----
            whT_cat = const_pool.tile([128, 4 * H], FP16, tag="whT")
            wsT_cat = const_pool.tile([128, 4 * H], FP16, tag="wsT")
            encT_cat = const_pool.tile([128, DC * S], FP16, tag="encT")
            qT_cat = const_pool.tile([128, DC * T], FP16, tag="qT")
            enc_cat = const_pool.tile([128, SB * H], FP32, tag="enc")
            vc_sb = const_pool.tile([128, NF * DC + 1], FP32, tag="vc")
            ident_sb = const_pool.tile([128, 128], FP32, tag="ident")

            def cat_load(dst, src_d, blocks):
                nc.sync.dma_start(
                    dst[:].rearrange("p (a j) -> p a j", a=blocks),
                    src_d.rearrange("(a p) j -> p a j", p=128),
                )

            cat_load(encT_cat, encT_d, DC)
            cat_load(whT_cat, whT_d, 4)
            cat_load(qT_cat, qT_d, DC)
            cat_load(wsT_cat, wsT_d, 4)
            cat_load(enc_cat, enc_d, SB)
            nc.sync.dma_start(vc_sb[:], vc_d[:])
            nc.sync.dma_start(ident_sb[:], ident_d[:])

            whT_sb = [whT_cat[:, H * i:H * (i + 1)] for i in range(4)]
            wsT_sb = [wsT_cat[:, H * i:H * (i + 1)] for i in range(4)]
            encT_sb = [encT_cat[:, S * i:S * (i + 1)] for i in range(DC)]
            qT_sb = [qT_cat[:, T * i:T * (i + 1)] for i in range(DC)]
            enc_sb = [enc_cat[:, H * i:H * (i + 1)] for i in range(SB)]

            import contextlib

            def _rep_ctx():
                if nrep == 1:
                    return contextlib.nullcontext()
                return tc.For_i(0, nrep, 1)

            def body():
                # ---- projections into x_cat [128, (eh dc0..3 | qs dc0..3)] ----
                x_cat = xcat_pool.tile([128, DC * W2], FP16, tag="x_cat")

                def project(dc, wT, xT, col0):
                    ps = ppsum.tile([128, xT[0].shape[-1]], FP32, tag="proj_ps")
                    n = xT[0].shape[-1]
                    for hc in range(4):
                        nc.tensor.matmul(
                            ps[:],
                            wT[hc][:, 128 * dc:128 * (dc + 1)],
                            xT[hc],
                            start=(hc == 0), stop=(hc == 3),
                        )
                    nc.scalar.copy(x_cat[:, col0:col0 + n], ps[:])

                for dc in range(DC):
                    project(dc, whT_sb, encT_sb, dc * S)
                for dc in range(DC):
                    project(dc, wsT_sb, qT_sb, DC * S + dc * T)

                # ---- base features on ACT (in-domain sin) ----
                NW = DC * W2
                sh = feat_pool.tile([128, NW], FP16, tag="sh")
                ch = feat_pool.tile([128, NW], FP16, tag="ch")
                nc.scalar.activation(sh[:], x_cat[:],
                                     mybir.ActivationFunctionType.Sin,
                                     scale=float(OMEGA / 2))
                nc.scalar.activation(ch[:], x_cat[:],
                                     mybir.ActivationFunctionType.Sin,
                                     bias=vc_sb[:, NF * DC:NF * DC + 1],
                                     scale=float(OMEGA / 2))

                # ---- harmonic ladder: DVE STT/TS + ACT squares ----
                def ftile(tag):
                    return feat_pool.tile([128, NW], FP16, name=tag, tag=tag)

                s = {}
                c = {}
                for k_ in range(1, 7):
                    s[k_] = ftile(f"s{k_}")
                for k_ in range(2, 7):
                    c[k_] = ftile(f"c{k_}")
                sq_ch = ftile("sq_ch")
                sq1 = ftile("sq1")
                sq2 = ftile("sq2")
                sq3 = ftile("sq3")
                chd = ftile("chd")
                c1d = ftile("c1d")
                p1 = ftile("p1")
                p2 = ftile("p2")
                p3 = ftile("p3")
                p4 = ftile("p4")

                SQ = mybir.ActivationFunctionType.Square
                tt = nc.vector.tensor_tensor
                ts = nc.vector.tensor_scalar

                # sin tiles exact; cos tiles stored at 2x (sigma=2), the
                # sigma^2 factor is divided out of vc on the host
                ts(chd[:], sh[:], 2.0, None, MULT)           # 2 sin(u)
                nc.scalar.activation(sq_ch[:], ch[:], SQ)
                ts(c1d[:], sq_ch[:], 4.0, 2.0, MULT, SUB)    # 2cos1
                tt(s[1][:], chd[:], ch[:], MULT)             # sin1
                nc.scalar.activation(sq1[:], s[1][:], SQ)
                ts(c[2][:], sq1[:], -4.0, 2.0, MULT, ADD)    # 2cos2
                tt(s[2][:], s[1][:], c1d[:], MULT)           # sin2
                nc.scalar.activation(sq2[:], s[2][:], SQ)
                ts(c[4][:], sq2[:], -4.0, 2.0, MULT, ADD)    # 2cos4
                tt(p1[:], s[2][:], c1d[:], MULT)             # sin3+sin1
                tt(s[3][:], p1[:], s[1][:], SUB)             # sin3
                tt(p2[:], c[2][:], c1d[:], MULT)             # 2cos3+2cos1
                tt(c[3][:], p2[:], c1d[:], SUB)              # 2cos3
                nc.scalar.activation(sq3[:], s[3][:], SQ)
                ts(c[6][:], sq3[:], -2.0, 1.0, MULT, ADD)    # cos6
                tt(s[4][:], s[2][:], c[2][:], MULT)          # sin2*2cos2 = sin4
                tt(p3[:], s[4][:], c1d[:], MULT)             # sin5+sin3
                tt(s[5][:], p3[:], s[3][:], SUB)             # sin5
                tt(p4[:], c[4][:], c1d[:], MULT)             # 2cos5+2cos3
                tt(c[5][:], p4[:], c[3][:], SUB)             # 2cos5
                tt(s[6][:], s[3][:], c[3][:], MULT)          # sin3*2cos3 = sin6

                # ---- folds + score matmuls, interleaved per (k, phase) ----
                score_ps = [spsum.tile([128, S], FP32, name=f"score{tb}", tag=f"score{tb}")
                            for tb in range(TB)]

                # pairings: folded eh-sin_k x qs-cos_k ; folded eh-cos_k x qs-sin_k
                c[1] = c1d     # cos features at sigma: c1,c2,c3,c4,c5 = 2x, c6 = 1x
                for ki in range(1, K_HARM + 1):
                    for ph, (ebase, qbase) in enumerate(((s, c), (c, s))):
                        kph = (ki - 1) * 2 + ph
                        fold = fold_pool.tile([128, DC * S], FP16, name=f"fold{kph}", tag=f"fold{kph}")
                        for dc in range(DC):
                            nc.gpsimd.tensor_scalar(
                                fold[:, dc * S:(dc + 1) * S],
                                ebase[ki][:, dc * S:(dc + 1) * S],
                                vc_sb[:, kph * DC + dc:kph * DC + dc + 1],
                                None, MULT)
                        for dc in range(DC):
                            for tb in range(TB):
                                nc.tensor.matmul(
                                    score_ps[tb][:],
                                    qbase[ki][:, DC * S + dc * T + 128 * tb:
                                              DC * S + dc * T + 128 * (tb + 1)],
                                    fold[:, dc * S:(dc + 1) * S],
                                    start=(kph == 0 and dc == 0),
                                    stop=(kph == NF - 1 and dc == DC - 1),
                                    skip_group_check=True,
                                )

                # ---- softmax via tanh (one table set, no exp) ----
                attn_sb = [post_pool.tile([128, S], FP32, name=f"attn{tb}", tag=f"attn{tb}")
                           for tb in range(TB)]
                rden = [post_pool.tile([128, 1], FP32, name=f"rden{tb}", tag=f"rden{tb}")
                        for tb in range(TB)]
                for tb in range(TB):
                    tt_sb = post_pool.tile([128, S], FP32, name=f"tt{tb}", tag=f"tt{tb}")
                    nc.scalar.activation(tt_sb[:], score_ps[tb][:],
                                         mybir.ActivationFunctionType.Tanh,
                                         scale=0.5)
                    dd = post_pool.tile([128, S], FP32, name=f"dd{tb}", tag=f"dd{tb}")
                    ts(dd[:], tt_sb[:], -1.0, 1.0, MULT, ADD)
                    rr = post_pool.tile([128, S], FP32, name=f"rr{tb}", tag=f"rr{tb}")
                    nc.vector.reciprocal_approx_fast(rr[:], dd[:])
                    den = post_pool.tile([128, 1], FP32, name=f"den{tb}", tag=f"den{tb}")
                    nc.vector.scalar_tensor_tensor(
                        attn_sb[tb][:], tt_sb[:], 1.0, rr[:], ADD, MULT,
                        accum_out=den[:])
                    nc.vector.reciprocal(rden[tb][:], den[:])

                # ---- transpose attn -> attnT ----
                attnT_sb = [post_pool.tile([128, T], FP32, name=f"attnT{sb}", tag=f"attnT{sb}")
                            for sb in range(SB)]
                for sb in range(SB):
                    at_ps = apsum.tile([128, T], FP32, tag="at_ps")
                    for tb in range(TB):
                        nc.tensor.transpose(
                            at_ps[:, 128 * tb:128 * (tb + 1)],
                            attn_sb[tb][:, 128 * sb:128 * (sb + 1)],
                            ident_sb[:],
                        )
                    nc.scalar.copy(attnT_sb[sb][:], at_ps[:])

                # ---- context ----
                for tb in range(TB):
                    ctx_ps = cpsum.tile([128, H], FP32, tag="ctx_ps")
                    for sb in range(SB):
                        nc.tensor.matmul(
                            ctx_ps[:],
                            attnT_sb[sb][:, 128 * tb:128 * (tb + 1)],
                            enc_sb[sb],
                            start=(sb == 0), stop=(sb == SB - 1),
                        )
                    ctx_sb = post_pool.tile([128, H], FP32, name=f"ctx{tb}", tag=f"ctx{tb}")
                    nc.scalar.activation(
                        ctx_sb[:], ctx_ps[:],
                        mybir.ActivationFunctionType.Identity,
                        scale=rden[tb][:])
                    nc.sync.dma_start(ctx_d[128 * tb:128 * (tb + 1), :], ctx_sb[:])

            with _rep_ctx():
                for _u in range(unroll):
                    body()

    import concourse.bacc as _bacc_mod
    _orig_tables = _bacc_mod.get_activation_tables

    def _filtered_tables(arch):
        tabs = _orig_tables(arch)
        need = {
            mybir.ActivationFunctionType.Sin,
            mybir.ActivationFunctionType.Tanh,
            mybir.ActivationFunctionType.Square,
            mybir.ActivationFunctionType.Identity,
            mybir.ActivationFunctionType.Copy,
        }
        full = {k: v for k, v in tabs.items() if need.issubset(v)}
        if not full:
            return tabs  # no single covering set: leave untouched
        # keep dict length/order (ids are positional); empty non-covering sets
        return {k: (v if k in full else set()) for k, v in tabs.items()}

    _bacc_mod.get_activation_tables = _filtered_tables
    try:
        nc.compile()
    finally:
        _bacc_mod.get_activation_tables = _orig_tables
    return nc


def make_in_maps(encoder_outputs, query, mask, W_h, W_s, v, T=T_FULL, S=S_FULL):
    B = encoder_outputs.shape[0]
    assert bool(np.asarray(mask).all()), "kernel assumes all-ones mask"
    whT = np.ascontiguousarray(np.asarray(W_h, np.float32).T.astype(np.float16))
    wsT = np.ascontiguousarray(np.asarray(W_s, np.float32).T.astype(np.float16))
    v32 = np.asarray(v, np.float32)
    NF = 2 * K_HARM
    DC = H // 128
    vc = np.zeros((128, NF * DC + 1), np.float32)
    vc[:, NF * DC] = np.pi / 2
    cos_sigma = {1: 2.0, 2: 2.0, 3: 2.0, 4: 2.0, 5: 2.0, 6: 1.0}
    for ki in range(1, K_HARM + 1):
        for ph in range(2):
            kph = (ki - 1) * 2 + ph
            # ph=0 pairs folded sin(eh) with cos(qs); ph=1 folded cos(eh)
            # with sin(qs). Either way exactly one cos tile (scale sigma)
            # enters the product, so divide by sigma once.
            sig = cos_sigma[ki]
            for dc in range(DC):
                vc[:, kph * DC + dc] = (C_K[ki - 1] / sig) * v32[dc * 128:(dc + 1) * 128]
    ident = np.eye(128, dtype=np.float32)
    in_maps = []
    for b in range(B):
        enc_b = np.asarray(encoder_outputs[b], np.float32)
        q_b = np.asarray(query[b], np.float32)
        in_maps.append({
            "encT16": np.ascontiguousarray(enc_b.T.astype(np.float16)),
            "qT16": np.ascontiguousarray(q_b.T.astype(np.float16)),
            "whT16": whT,
            "wsT16": wsT,
            "enc32": np.ascontiguousarray(enc_b),
            "vc32": vc,
            "ident": ident,
        })
    return in_maps


_PROGRAM_CACHE = {}


def kernel(encoder_outputs, query, mask, W_h, W_s, v):
    from concourse.bass_utils import run_bass_kernel_spmd

    B = encoder_outputs.shape[0]
    assert B == N_CORES
    key = (T_FULL, S_FULL, N_CORES)
    if key not in _PROGRAM_CACHE:
        _PROGRAM_CACHE[key] = build_program()
    nc = _PROGRAM_CACHE[key]
    in_maps = make_in_maps(encoder_outputs, query, mask, W_h, W_s, v)
    res = run_bass_kernel_spmd(nc, in_maps, list(range(N_CORES)))
    out = np.stack([res.results[b]["ctx"] for b in range(B)], axis=0)
    return out.astype(np.float32)


# revision 13
# speedup vs baseline: 3.6021x; 3.6021x over previous
"""Bahdanau additive attention on 8 Trainium2 NeuronCores.

reference:
    eh = enc @ W_h.T            [B,S,H]
    qs = q   @ W_s.T            [B,T,H]
    score[b,t,s] = sum_h v[h] * tanh(eh[b,s,h] + qs[b,t,h])
    attn = softmax_s(score); ctx[b,t,:] = sum_s attn[b,t,s] * enc[b,s,:]

Sharding: data-parallel over batch B=8, one batch per NeuronCore.

Algorithm (replaces the 33.5M-element/core direct tanh evaluation):
separable Fourier approximation
    tanh(x) ~= sum_{k=1..6} c_k sin(k*w*x),  w = pi/9
so with angle addition the score becomes a plain contraction
    score[t,s] = sum_h sum_k (c_k v_h sin_k(eh)) cos_k(qs)
                            + (c_k v_h cos_k(eh)) sin_k(qs)
Per-core device program:
  - PE: ehT/qsT projections (fp16 weights, fp32 PSUM)
  - ACT: base pair sin(w/2 x), cos(w/2 x) (in the [-pi,pi] spline domain)
    plus squares for the double-angle cosines
  - DVE: Chebyshev/double-angle ladder for harmonics k=1..6 (fp16),
    then per-(k,phase,dc) folds multiplying by c_k*v (per-partition scalars)
  - PE: score matmul, contraction (h,k,phase) = 12*512 on partitions
  - softmax without exp: t = tanh(score/2) on ACT, e^z = (1+t)/(1-t) via
    reciprocal_approx_fast on DVE; normalization folded into the epilogue
  - PE transposes attn, context matmul against enc natural layout
"""

import sys

try:
    import concourse.bass as bass  # noqa: F401
except ImportError:  # pragma: no cover
    sys.path.insert(0, "/opt/trn_rl_repo")

import numpy as np

import concourse.bass as bass
import concourse.bacc as bacc
import concourse.mybir as mybir
from concourse import tile

FP32 = mybir.dt.float32
FP16 = mybir.dt.float16

N_CORES = 8
H = 512
T_FULL = 256
S_FULL = 256

K_HARM = 6
L_PER = 9.0
OMEGA = np.pi / L_PER


def design_coeffs():
    """Weighted LSQ fit of tanh(x) by sum_k c_k sin(k*OMEGA*x)."""
    xs = np.linspace(0, L_PER, 40001)
    wgt = np.exp(-xs ** 2 / 4.0)
    wgt = np.maximum(wgt, 1e-3)
    wgt[xs > 0.97 * L_PER] *= 0.05
    A = np.stack([np.sin(k * OMEGA * xs) for k in range(1, K_HARM + 1)], 1)
    Wt = np.sqrt(wgt)
    c, *_ = np.linalg.lstsq(A * Wt[:, None], np.tanh(xs) * Wt, rcond=None)
    return c


C_K = design_coeffs()

MULT = mybir.AluOpType.mult
ADD = mybir.AluOpType.add
SUB = mybir.AluOpType.subtract


def build_program(T=T_FULL, S=S_FULL, n_cores=N_CORES, nrep=1, debug=False,
                  unroll=1):
    assert H % 128 == 0 and T % 128 == 0 and S % 128 == 0
    DC = H // 128
    TB = T // 128
    SB = S // 128
    NF = 2 * K_HARM          # folded feature tiles (k, sin/cos)
    W2 = S + T               # combined free width of one feature tile

    nc = bacc.Bacc("TRN2", target_bir_lowering=False, debug=debug,
                   num_devices=n_cores)

    encT_d = nc.dram_tensor("encT16", [H, S], FP16, kind="ExternalInput")
    qT_d = nc.dram_tensor("qT16", [H, T], FP16, kind="ExternalInput")
    whT_d = nc.dram_tensor("whT16", [H, H], FP16, kind="ExternalInput")
    wsT_d = nc.dram_tensor("wsT16", [H, H], FP16, kind="ExternalInput")
    enc_d = nc.dram_tensor("enc32", [S, H], FP32, kind="ExternalInput")
    vc_d = nc.dram_tensor("vc32", [128, NF * DC + 1], FP32, kind="ExternalInput")
    ident_d = nc.dram_tensor("ident", [128, 128], FP32, kind="ExternalInput")
    ctx_d = nc.dram_tensor("ctx", [T, H], FP32, kind="ExternalOutput")

    with tile.TileContext(nc) as tc:
        with (
            tc.tile_pool(name="const", bufs=1) as const_pool,
            tc.tile_pool(name="xcat", bufs=2) as xcat_pool,
            tc.tile_pool(name="feat", bufs=1) as feat_pool,
            tc.tile_pool(name="fold", bufs=1) as fold_pool,
            tc.tile_pool(name="post", bufs=2) as post_pool,
            tc.tile_pool(name="ppsum", bufs=2, space=bass.MemorySpace.PSUM) as ppsum,
            tc.tile_pool(name="spsum", bufs=2, space=bass.MemorySpace.PSUM) as spsum,
            tc.tile_pool(name="apsum", bufs=1, space=bass.MemorySpace.PSUM) as apsum,
            tc.tile_pool(name="cpsum", bufs=1, space=bass.MemorySpace.PSUM) as cpsum,
        ):
            # ---- constants / inputs (outside the timed loop) ----
            whT_cat = const_pool.tile([128, 4 * H], FP16, tag="whT")
            wsT_cat = const_pool.tile([128, 4 * H], FP16, tag="wsT")
            encT_cat = const_pool.tile([128, DC * S], FP16, tag="encT")
            qT_cat = const_pool.tile([128, DC * T], FP16, tag="qT")
            enc_cat = const_pool.tile([128, SB * H], FP32, tag="enc")
            vc_sb = const_pool.tile([128, NF * DC + 1], FP32, tag="vc")
            ident_sb = const_pool.tile([128, 128], FP32, tag="ident")

            def cat_load(dst, src_d, blocks):
                nc.sync.dma_start(
                    dst[:].rearrange("p (a j) -> p a j", a=blocks),
                    src_d.rearrange("(a p) j -> p a j", p=128),
                )

            cat_load(encT_cat, encT_d, DC)
            cat_load(whT_cat, whT_d, 4)
            cat_load(qT_cat, qT_d, DC)
            cat_load(wsT_cat, wsT_d, 4)
            cat_load(enc_cat, enc_d, SB)
            nc.sync.dma_start(vc_sb[:], vc_d[:])
            nc.sync.dma_start(ident_sb[:], ident_d[:])

            whT_sb = [whT_cat[:, H * i:H * (i + 1)] for i in range(4)]
            wsT_sb = [wsT_cat[:, H * i:H * (i + 1)] for i in range(4)]
            encT_sb = [encT_cat[:, S * i:S * (i + 1)] for i in range(DC)]
            qT_sb = [qT_cat[:, T * i:T * (i + 1)] for i in range(DC)]
            enc_sb = [enc_cat[:, H * i:H * (i + 1)] for i in range(SB)]

            import contextlib

            def _rep_ctx():
                if nrep == 1:
                    return contextlib.nullcontext()
                return tc.For_i(0, nrep, 1)

            def body():
                # ---- projections into x_cat [128, (eh dc0..3 | qs dc0..3)] ----
                x_cat = xcat_pool.tile([128, DC * W2], FP16, tag="x_cat")

                def project(dc, wT, xT, col0):
                    ps = ppsum.tile([128, xT[0].shape[-1]], FP32, tag="proj_ps")
                    n = xT[0].shape[-1]
                    for hc in range(4):
                        nc.tensor.matmul(
                            ps[:],
                            wT[hc][:, 128 * dc:128 * (dc + 1)],
                            xT[hc],
                            start=(hc == 0), stop=(hc == 3),
                        )
                    nc.scalar.copy(x_cat[:, col0:col0 + n], ps[:])

                for dc in range(DC):
                    project(dc, whT_sb, encT_sb, dc * S)
                for dc in range(DC):
                    project(dc, wsT_sb, qT_sb, DC * S + dc * T)

                # ---- base features on ACT (in-domain sin) ----
                NW = DC * W2
                sh = feat_pool.tile([128, NW], FP16, tag="sh")
                ch = feat_pool.tile([128, NW], FP16, tag="ch")
                nc.scalar.activation(sh[:], x_cat[:],
                                     mybir.ActivationFunctionType.Sin,
                                     scale=float(OMEGA / 2))
                nc.scalar.activation(ch[:], x_cat[:],
                                     mybir.ActivationFunctionType.Sin,
                                     bias=vc_sb[:, NF * DC:NF * DC + 1],
                                     scale=float(OMEGA / 2))

                # ---- harmonic ladder: DVE STT/TS + ACT squares ----
                def ftile(tag):
                    return feat_pool.tile([128, NW], FP16, name=tag, tag=tag)

                s = {}
                c = {}
                for k_ in range(1, 7):
                    s[k_] = ftile(f"s{k_}")
                for k_ in range(2, 7):
                    c[k_] = ftile(f"c{k_}")
                sq_ch = ftile("sq_ch")
                sq1 = ftile("sq1")
                sq2 = ftile("sq2")
                sq3 = ftile("sq3")
                chd = ftile("chd")
                c1d = ftile("c1d")
                p1 = ftile("p1")
                p2 = ftile("p2")
                p3 = ftile("p3")
                p4 = ftile("p4")

                SQ = mybir.ActivationFunctionType.Square
                tt = nc.vector.tensor_tensor
                ts = nc.vector.tensor_scalar

                # sin tiles exact; cos tiles stored at 2x (sigma=2), the
                # sigma^2 factor is divided out of vc on the host
                ts(chd[:], sh[:], 2.0, None, MULT)           # 2 sin(u)
                nc.scalar.activation(sq_ch[:], ch[:], SQ)
                ts(c1d[:], sq_ch[:], 4.0, 2.0, MULT, SUB)    # 2cos1
                tt(s[1][:], chd[:], ch[:], MULT)             # sin1
                nc.scalar.activation(sq1[:], s[1][:], SQ)
                ts(c[2][:], sq1[:], -4.0, 2.0, MULT, ADD)    # 2cos2
                tt(s[2][:], s[1][:], c1d[:], MULT)           # sin2
                nc.scalar.activation(sq2[:], s[2][:], SQ)
                ts(c[4][:], sq2[:], -4.0, 2.0, MULT, ADD)    # 2cos4
                ts(p1[:], sq1[:], -4.0, 3.0, MULT, ADD)      # 3-4sin1^2
                tt(s[3][:], s[1][:], p1[:], MULT)            # sin3
                ts(p2[:], c[2][:], 1.0, 1.0, MULT, SUB)      # 2cos2-1
                tt(c[3][:], c1d[:], p2[:], MULT)             # 2cos3
                nc.scalar.activation(sq3[:], s[3][:], SQ)
                ts(c[6][:], sq3[:], -2.0, 1.0, MULT, ADD)    # cos6
                tt(s[4][:], s[2][:], c[2][:], MULT)          # sin2*2cos2 = sin4
                tt(p3[:], s[4][:], c1d[:], MULT)             # sin5+sin3
                tt(s[5][:], p3[:], s[3][:], SUB)             # sin5
                tt(p4[:], c[4][:], c1d[:], MULT)             # 2cos5+2cos3
                tt(c[5][:], p4[:], c[3][:], SUB)             # 2cos5
                tt(s[6][:], s[3][:], c[3][:], MULT)          # sin3*2cos3 = sin6

                # ---- folds + score matmuls, interleaved per (k, phase) ----
                score_ps = [spsum.tile([128, S], FP32, name=f"score{tb}", tag=f"score{tb}")
                            for tb in range(TB)]

                # pairings: folded eh-sin_k x qs-cos_k ; folded eh-cos_k x qs-sin_k
                c[1] = c1d     # cos features at sigma: c1,c2,c3,c4,c5 = 2x, c6 = 1x
                for ki in range(1, K_HARM + 1):
                    for ph, (ebase, qbase) in enumerate(((s, c), (c, s))):
                        kph = (ki - 1) * 2 + ph
                        fold = fold_pool.tile([128, DC * S], FP16, name=f"fold{kph}", tag=f"fold{kph}")
                        for dc in range(DC):
                            ts(fold[:, dc * S:(dc + 1) * S],
                               ebase[ki][:, dc * S:(dc + 1) * S],
                               vc_sb[:, kph * DC + dc:kph * DC + dc + 1],
                               None, MULT)
                        for dc in range(DC):
                            for tb in range(TB):
                                nc.tensor.matmul(
                                    score_ps[tb][:],
                                    qbase[ki][:, DC * S + dc * T + 128 * tb:
                                              DC * S + dc * T + 128 * (tb + 1)],
                                    fold[:, dc * S:(dc + 1) * S],
                                    start=(kph == 0 and dc == 0),
                                    stop=(kph == NF - 1 and dc == DC - 1),
                                    skip_group_check=True,
                                )

                # ---- softmax via tanh (one table set, no exp) ----
                attn_sb = [post_pool.tile([128, S], FP32, name=f"attn{tb}", tag=f"attn{tb}")
                           for tb in range(TB)]
                rden = [post_pool.tile([128, 1], FP32, name=f"rden{tb}", tag=f"rden{tb}")
                        for tb in range(TB)]
                for tb in range(TB):
                    tt_sb = post_pool.tile([128, S], FP32, name=f"tt{tb}", tag=f"tt{tb}")
                    nc.scalar.activation(tt_sb[:], score_ps[tb][:],
                                         mybir.ActivationFunctionType.Tanh,
                                         scale=0.5)
                    dd = post_pool.tile([128, S], FP32, name=f"dd{tb}", tag=f"dd{tb}")
                    ts(dd[:], tt_sb[:], -1.0, 1.0, MULT, ADD)
                    rr = post_pool.tile([128, S], FP32, name=f"rr{tb}", tag=f"rr{tb}")
                    nc.vector.reciprocal_approx_fast(rr[:], dd[:])
                    den = post_pool.tile([128, 1], FP32, name=f"den{tb}", tag=f"den{tb}")
                    nc.vector.scalar_tensor_tensor(
                        attn_sb[tb][:], tt_sb[:], 1.0, rr[:], ADD, MULT,
                        accum_out=den[:])
                    nc.vector.reciprocal(rden[tb][:], den[:])

                # ---- transpose attn -> attnT ----
                attnT_sb = [post_pool.tile([128, T], FP32, name=f"attnT{sb}", tag=f"attnT{sb}")
                            for sb in range(SB)]
                for sb in range(SB):
                    at_ps = apsum.tile([128, T], FP32, tag="at_ps")
                    for tb in range(TB):
                        nc.tensor.transpose(
                            at_ps[:, 128 * tb:128 * (tb + 1)],
                            attn_sb[tb][:, 128 * sb:128 * (sb + 1)],
                            ident_sb[:],
                        )
                    nc.scalar.copy(attnT_sb[sb][:], at_ps[:])

                # ---- context ----
                for tb in range(TB):
                    ctx_ps = cpsum.tile([128, H], FP32, tag="ctx_ps")
                    for sb in range(SB):
                        nc.tensor.matmul(
                            ctx_ps[:],
                            attnT_sb[sb][:, 128 * tb:128 * (tb + 1)],
                            enc_sb[sb],
                            start=(sb == 0), stop=(sb == SB - 1),
                        )
                    ctx_sb = post_pool.tile([128, H], FP32, name=f"ctx{tb}", tag=f"ctx{tb}")
                    nc.scalar.activation(
                        ctx_sb[:], ctx_ps[:],
                        mybir.ActivationFunctionType.Identity,
                        scale=rden[tb][:])
                    nc.sync.dma_start(ctx_d[128 * tb:128 * (tb + 1), :], ctx_sb[:])

            with _rep_ctx():
                for _u in range(unroll):
                    body()

    import concourse.bacc as _bacc_mod
    _orig_tables = _bacc_mod.get_activation_tables

    def _filtered_tables(arch):
        tabs = _orig_tables(arch)
        need = {
            mybir.ActivationFunctionType.Sin,
            mybir.ActivationFunctionType.Tanh,
            mybir.ActivationFunctionType.Square,
            mybir.ActivationFunctionType.Identity,
            mybir.ActivationFunctionType.Copy,
        }
        full = {k: v for k, v in tabs.items() if need.issubset(v)}
        if not full:
            return tabs  # no single covering set: leave untouched
        # keep dict length/order (ids are positional); empty non-covering sets
        return {k: (v if k in full else set()) for k, v in tabs.items()}

    _bacc_mod.get_activation_tables = _filtered_tables
    try:
        nc.compile()
    finally:
        _bacc_mod.get_activation_tables = _orig_tables
    return nc


def make_in_maps(encoder_outputs, query, mask, W_h, W_s, v, T=T_FULL, S=S_FULL):
    B = encoder_outputs.shape[0]
    assert bool(np.asarray(mask).all()), "kernel assumes all-ones mask"
    whT = np.ascontiguousarray(np.asarray(W_h, np.float32).T.astype(np.float16))
    wsT = np.ascontiguousarray(np.asarray(W_s, np.float32).T.astype(np.float16))
    v32 = np.asarray(v, np.float32)
    NF = 2 * K_HARM
    DC = H // 128
    vc = np.zeros((128, NF * DC + 1), np.float32)
    vc[:, NF * DC] = np.pi / 2
    cos_sigma = {1: 2.0, 2: 2.0, 3: 2.0, 4: 2.0, 5: 2.0, 6: 1.0}
    for ki in range(1, K_HARM + 1):
        for ph in range(2):
            kph = (ki - 1) * 2 + ph
            # ph=0 pairs folded sin(eh) with cos(qs); ph=1 folded cos(eh)
            # with sin(qs). Either way exactly one cos tile (scale sigma)
            # enters the product, so divide by sigma once.
            sig = cos_sigma[ki]
            for dc in range(DC):
                vc[:, kph * DC + dc] = (C_K[ki - 1] / sig) * v32[dc * 128:(dc + 1) * 128]
    ident = np.eye(128, dtype=np.float32)
    in_maps = []
    for b in range(B):
        enc_b = np.asarray(encoder_outputs[b], np.float32)
        q_b = np.asarray(query[b], np.float32)
        in_maps.append({
            "encT16": np.ascontiguousarray(enc_b.T.astype(np.float16)),
            "qT16": np.ascontiguousarray(q_b.T.astype(np.float16)),
            "whT16": whT,
            "wsT16": wsT,
            "enc32": np.ascontiguousarray(enc_b),
            "vc32": vc,
            "ident": ident,
        })
    return in_maps


_PROGRAM_CACHE = {}


def kernel(encoder_outputs, query, mask, W_h, W_s, v):
    from concourse.bass_utils import run_bass_kernel_spmd

    B = encoder_outputs.shape[0]
    assert B == N_CORES
    key = (T_FULL, S_FULL, N_CORES)
    if key not in _PROGRAM_CACHE:
        _PROGRAM_CACHE[key] = build_program()
    nc = _PROGRAM_CACHE[key]
    in_maps = make_in_maps(encoder_outputs, query, mask, W_h, W_s, v)
    res = run_bass_kernel_spmd(nc, in_maps, list(range(N_CORES)))
    out = np.stack([res.results[b]["ctx"] for b in range(B)], axis=0)
    return out.astype(np.float32)
